# revision 12
# baseline (speedup 1.0000x reference)
"""Trainium2 Bass kernel for nn_CircumpunctAttention.

Full inputs in, full output out. Data-parallel over batch (2) x
tensor-parallel over heads (4 head-groups of 4 heads) = 8 NeuronCores.

v2 design: ACT(exp)-bound streaming pipeline.
  - fp8e4 DoubleRow projections (x, Wq/Wk/Wv scaled x32 host-side) and
    emerge (oT, chamber-folded We): K=256 per matmul, 2 fp8 MACs/cell.
  - Scores run as row-tiled concurrent head pairs (K=64 at array rows 0-63
    and 64-127), bf16, one [128,512] chunk per head per key-tile.
  - PSUM is split into a 6-bank ring of [128,512] f32 slots (scores->exp,
    projection and emerge outputs) plus 2 attnV accumulator banks.  PSUM
    accumulation groups are bank-granular, so every concurrent group owns
    a full bank.  exp consumes runs of up to 3 ring slots in one
    activation (1536-wide) to amortize the ~352-cycle ACT startup.
  - attnV per head (M=96: 64 ch + denominator-ones + pad) accumulates in
    po bank 6 (head A) / bank 7 (head B); the [65,512] result is drained
    to SBUF right away so the bank frees for the next sweep.
  - Softmax denominator = ones column in va; normalization via DRAM-bounce
    broadcast + custom-DVE fast reciprocal (must run at partition base 0).
  - Emission is software-pipelined: attnV lags scores by LAG steps so the
    in-order PE queue never sits ahead of the exp results it waits on.
    Ring-consuming side work (projections, emerge) is deferred to exp-run
    boundaries so runs keep contiguous slots.

Scaling bookkeeping (powers of 2, folded into free affine slots):
  Wq,Wk scaled x32 -> raw scores carry 32*32*8 = 2^13 -> exp scale 2^-13.
  Wv scaled x32 -> va, oT numerator carry x32; denominator unscaled ones
  -> oT = 32*o_true, stored fp8e4 (sigma~2).  We chamber-folded then x128
  -> emerge psum = 4096*out -> 2^-12 folded into the gpsimd staging copy.
"""

import math
from contextlib import ExitStack
import numpy as np

# ---------------------------------------------------------------- constants
P = 128          # partitions
T = 2048         # sequence length
D = 1024         # model dim
H = 16           # total heads
DH = 64          # head dim
HC = 4           # heads per core
C = HC * DH      # channels per core (256)
KT = D // P      # 8 contraction tiles over model dim
TT = T // P      # 16 key tiles of 128
MT = C // P      # 2 partition tiles over per-core channels
QB = 512         # query block (ring slot width)
NQB = T // QB    # 4 query blocks
NCORES = 8
SCALE = 8.0      # sqrt(dh * conv_factor)
VAM = 96         # attnV stationary cols: 64 ch + 1 ones + pad to 96

S_W = 1.0        # weights stay bf16, unscaled
S_WE = 1.0       # We stays bf16 (fp8 was too lossy)
EXP_SCALE = 1.0 / SCALE                        # 1/8
OUT_DESCALE = 1.0

RUN = 3          # exp ring slots per activation call (1536 wide)
LAG = 2          # attnV lags scores by this many kt steps

CFG = {
    "dve_exp_every": 0,   # 0 = all exp on ACT; N>0 = every Nth run on DVE
}

LAST_EXEC_NS = None
_CACHE = {}


def _np_dt(name):
    import ml_dtypes
    if name == "bfloat16":
        return np.dtype(ml_dtypes.bfloat16)
    if name == "float8e4":
        return np.dtype(ml_dtypes.float8_e4m3)
    return np.dtype(name)


# ------------------------------------------------------------- DVE fast exp
# exp(x) ~= (1 + x/4096)^4096 via 12 chained squarings split across two
# custom-DVE instructions (v3 pipeline has 8 ALU stages).  Max rel err
# ~s^2/8192 (0.6% at |s|=7), well inside the softmax tolerance.
def _register_dve_exp():
    import concourse.dve_ops as dve_ops
    from concourse.dve_ops import DveOp
    from concourse.dve_spec import Spec, Src0, C0, One, sq
    import re

    if "EXP_POW64_A_ANT" in dve_ops._SUB_OPCODE_FOR_NAME:
        return [op for op in dve_ops.OPS
                if op.name in ("EXP_POW64_A_ANT", "POW64_ANT")]

    def ref_a(in0, in1, c0, c1, c2):
        u = (1.0 + in0.astype(np.float32) * np.float32(c0)).astype(np.float32)
        for _ in range(6):
            u = (u * u).astype(np.float32)
        return u

    def ref_b(in0, in1, c0, c1, c2):
        u = in0.astype(np.float32)
        for _ in range(6):
            u = (u * u).astype(np.float32)
        return u

    body_a = Src0 * C0 + One
    for _ in range(6):
        body_a = sq(body_a)
    body_b = sq(Src0)
    for _ in range(5):
        body_b = sq(body_b)

    ops = [
        DveOp("EXP_POW64_A_ANT", Spec(body=body_a, reference=ref_a),
              subdim=False, uops_sha={}),
        DveOp("POW64_ANT", Spec(body=body_b, reference=ref_b),
              subdim=False, uops_sha={}),
    ]
    for op in ops:
        dve_ops.OPS.append(op)
        dve_ops.CUSTOM_DVE_SPECS[op.name] = op.spec
        dve_ops._SUB_OPCODE_FOR_NAME[op.name] = (
            dve_ops._CUSTOM_DVE_ROW_BASE + len(dve_ops.OPS) - 1)
        ver = "v3"  # TRN2
        try:
            op.compile(ver)
        except ValueError as e:
            m = re.search(r"%s: ([0-9a-f]+)" % ver, str(e))
            op.uops_sha[ver] = m.group(1)
            op.compile(ver)
    return ops


def build_nc(cfg=CFG):
    """Build + compile the single-core SPMD program."""
    import concourse.mybir as mybir
    import concourse.tile as tile
    from concourse import bacc

    dve_every = cfg["dve_exp_every"]
    if dve_every:
        EXPA, EXPB = _register_dve_exp()

    dt = mybir.dt
    f32 = dt.float32
    bf16 = dt.bfloat16
    fp8 = dt.float8e4
    DR = mybir.MatmulPerfMode.DoubleRow
    Exp = mybir.ActivationFunctionType.Exp

    nc = bacc.Bacc("TRN2", target_bir_lowering=False, debug=False,
                   enable_asserts=False)

    xt = nc.dram_tensor("xt", [D, T], bf16, kind="ExternalInput").ap()
    wq = nc.dram_tensor("wq", [D, C], bf16, kind="ExternalInput").ap()
    wk = nc.dram_tensor("wk", [D, C], bf16, kind="ExternalInput").ap()
    wv = nc.dram_tensor("wv", [D, C], bf16, kind="ExternalInput").ap()
    we = nc.dram_tensor("we", [C, D], bf16, kind="ExternalInput").ap()
    out = nc.dram_tensor("out", [T, D], f32, kind="ExternalOutput").ap()

    with tile.TileContext(nc) as tc, ExitStack() as ctx:
        cp = ctx.enter_context(tc.tile_pool(name="const", bufs=1))
        ringp = ctx.enter_context(tc.tile_pool(name="ring", bufs=1,
                                               space="PSUM"))
        pop = ctx.enter_context(tc.tile_pool(name="pop", bufs=1,
                                             space="PSUM"))
        nrm = ctx.enter_context(tc.tile_pool(name="nrm", bufs=4))
        drp = ctx.enter_context(tc.tile_pool(name="drb", bufs=4,
                                             space="DRAM"))
        obp = ctx.enter_context(tc.tile_pool(name="ob", bufs=2))
        ascr = ctx.enter_context(tc.tile_pool(name="ascr", bufs=2))

        xT_sb = cp.tile([P, KT, T], bf16)
        wq_sb = cp.tile([P, KT, C], bf16)
        wk_sb = cp.tile([P, KT, C], bf16)
        wv_sb = cp.tile([P, KT, C], bf16)
        we_sb = cp.tile([P, MT, D], bf16)
        qT_sb = cp.tile([P, MT, T], bf16)
        kT_sb = cp.tile([P, MT, T], bf16)
        va_sb = cp.tile([P, TT, HC, VAM], bf16)
        oT_sb = cp.tile([P, MT, T], bf16)
        pr_sb = cp.tile([P, 12, QB], bf16)      # p ring (2 PSUM rings deep)
        scr = cp.tile([P, 512], bf16)           # warmup scratch

        ring = ringp.tile([P, 6, QB], f32)      # PSUM banks 0-5
        po = pop.tile([P, 2, QB], f32)          # PSUM banks 6 (A), 7 (B)

        nc.vector.memset(va_sb, 0.0)
        nc.vector.memset(va_sb[:, :, :, DH:DH + 1], 1.0)
        nc.vector.memset(scr, 0.0)

        # ---- warmup: keep the PE HAM window busy during the input DMAs
        for i in range(8):
            nc.tensor.matmul(po[:, 0, :], lhsT=scr[:, 0:P],
                             rhs=scr[:, 0:QB], start=True, stop=True)

        # ---- input DMAs.  x arrives in 4 column chunks so the first
        # projections can start ~2.5us in.
        nc.sync.dma_start(out=wk_sb, in_=wk.rearrange("(k p) c -> p k c", p=P))
        nc.sync.dma_start(out=wq_sb, in_=wq.rearrange("(k p) c -> p k c", p=P))
        for xc in range(4):
            cs = slice(xc * 512, (xc + 1) * 512)
            nc.sync.dma_start(
                out=xT_sb[:, :, cs],
                in_=xt[:, cs].rearrange("(k p) t -> p k t", p=P))
        nc.gpsimd.dma_start(out=wv_sb,
                            in_=wv.rearrange("(k p) c -> p k c", p=P))
        nc.gpsimd.dma_start(out=we_sb,
                            in_=we.rearrange("(m p) d -> p m d", p=P))

        # ------------------------------------------------ ring bookkeeping
        pos = [0]          # next PSUM ring slot (mod 6)
        ppos = [0]         # next p-ring slot (mod 12)
        pend = []          # pending exp chunks [(slot, pslot)]
        nrun = [0]         # completed exp runs (for DVE cadence)

        def ring_slot():
            s = pos[0] % 6
            pos[0] += 1
            return s

        def contig_pieces(items):
            """Split [(s, pp)] into maximal pieces contiguous in BOTH."""
            out_p = []
            start = 0
            for i in range(1, len(items)):
                if (items[i][0] != items[i - 1][0] + 1
                        or items[i][1] != items[i - 1][1] + 1):
                    out_p.append(items[start:i])
                    start = i
            out_p.append(items[start:])
            return out_p

        def flush_exp():
            if not pend:
                return
            use_dve = dve_every and (nrun[0] % dve_every == dve_every - 1)
            for piece in contig_pieces(pend):
                s0, pp0 = piece[0]
                n = len(piece)
                src = ring[:, s0:s0 + n, :]
                dst = pr_sb[:, pp0:pp0 + n, :]
                if use_dve:
                    a_t = ascr.tile([P, RUN, QB], f32, tag="a")
                    nc.vector._custom_dve(
                        EXPA, out=a_t[:, 0:n, :], in0=src,
                        s0=EXP_SCALE / 4096.0, s1=0.0, imm2=0.0)
                    nc.vector._custom_dve(
                        EXPB, out=dst, in0=a_t[:, 0:n, :],
                        s0=0.0, s1=0.0, imm2=0.0)
                else:
                    nc.scalar.activation(dst, src, Exp, scale=EXP_SCALE)
            nrun[0] += 1
            pend.clear()

        def exp_chunk():
            s = pos[0] % 6
            pos[0] += 1
            pp = ppos[0] % 12
            ppos[0] += 1
            pend.append((s, pp))
            return s, pp

        def maybe_flush():
            if len(pend) >= RUN:
                flush_exp()

        # ------------------------------------------------ work units
        def proj_qk(w_sb, dst_sb, m, c0):
            """dst[:, m, c0:c0+512] = (W[:, m-block]^T x)[:, c0:c0+512]"""
            s = ring_slot()
            cs = slice(c0, c0 + QB)
            for j in range(KT):
                nc.tensor.matmul(
                    ring[:, s, :],
                    lhsT=w_sb[:, j, m * P:(m + 1) * P],
                    rhs=xT_sb[:, j, cs],
                    start=(j == 0), stop=(j == KT - 1))
            nc.vector.tensor_copy(dst_sb[:, m, cs], ring[:, s, :])

        def proj_v(t):
            s = ring_slot()
            for j in range(KT):
                nc.tensor.matmul(
                    ring[:, s, 0:C],
                    lhsT=xT_sb[:, j, t * P:(t + 1) * P],
                    rhs=wv_sb[:, j, :],
                    start=(j == 0), stop=(j == KT - 1))
            nc.vector.tensor_copy(
                va_sb[:, t, :, 0:DH],
                ring[:, s, 0:C].rearrange("p (h d) -> p h d", h=HC))

        def scores_pair(m, qb, kt):
            sA, ppA = exp_chunk()
            sB, ppB = exp_chunk()
            ks = slice(kt * P, (kt + 1) * P)
            qs = slice(qb * QB, (qb + 1) * QB)
            nc.tensor.matmul(ring[:, sA, :], lhsT=kT_sb[0:DH, m, ks],
                             rhs=qT_sb[0:DH, m, qs], start=True, stop=True)
            nc.tensor.matmul(ring[:, sB, :], lhsT=kT_sb[DH:P, m, ks],
                             rhs=qT_sb[DH:P, m, qs], start=True, stop=True)
            maybe_flush()
            return ppA, ppB

        def attnv(pair, kt, ppA, ppB):
            for h2, pp in ((0, ppA), (1, ppB)):
                nc.tensor.matmul(
                    po[0:VAM, h2, :],
                    lhsT=va_sb[:, kt, 2 * pair + h2, :],
                    rhs=pr_sb[:, pp, :],
                    start=(kt == 0), stop=(kt == TT - 1))

        def normalize(qb, pair):
            m = pair
            qs = slice(qb * QB, (qb + 1) * QB)
            for h2 in (0, 1):
                # drain the accumulator bank quickly so the next sweep can
                # restart attnV without waiting for the DRAM bounce
                u = nrm.tile([DH + 1, QB], f32, tag="u")
                nc.vector.tensor_copy(u, po[0:DH + 1, h2, :])
                rd = drp.tile([1, QB], f32, tag="rd")
                nc.sync.dma_start(out=rd, in_=u[DH:DH + 1, :])
                lbc = nrm.tile([DH, QB], f32, tag="lbc")
                nc.sync.dma_start(out=lbc, in_=rd.to_broadcast((DH, QB)))
                rbc = nrm.tile([DH, QB], f32, tag="rbc")
                nc.vector.reciprocal_approx_fast(rbc, lbc)
                if h2 == 0:
                    nc.vector.tensor_mul(oT_sb[0:DH, m, qs], u[0:DH, :], rbc)
                else:
                    st = nrm.tile([DH, QB], bf16, tag="st")
                    nc.vector.tensor_mul(st, u[0:DH, :], rbc)
                    nc.gpsimd.dma_start(out=oT_sb[DH:P, m, qs], in_=st)

        def emerge_t(t):
            slots = [ring_slot(), ring_slot()]
            ts = slice(t * P, (t + 1) * P)
            for half in range(2):
                for m in range(MT):
                    nc.tensor.matmul(
                        ring[:, slots[half], :],
                        lhsT=oT_sb[:, m, ts],
                        rhs=we_sb[:, m, half * QB:(half + 1) * QB],
                        start=(m == 0), stop=(m == MT - 1))
            ob = obp.tile([P, D], f32, tag="ob")
            for piece in contig_pieces([(s, s) for s in slots]):
                s0, n = piece[0][0], len(piece)
                i0 = slots.index(s0)
                nc.vector.tensor_copy(
                    ob[:, i0 * QB:(i0 + n) * QB],
                    ring[:, s0:s0 + n, :].rearrange("p a b -> p (a b)"))
            nc.sync.dma_start(out=out[ts, :], in_=ob)

        # ------------------------------------------------ emission schedule
        sweeps = [(qb, pair) for qb in range(NQB) for pair in range(2)]

        prework = {(si, kt): [] for si in range(len(sweeps))
                   for kt in range(TT)}

        def add(si, kt, fn):
            prework[(si, kt)].append(fn)

        # k projections: m0 kb0 upfront; rest early in sweeps 0/1
        for j, (si, kt) in enumerate([(0, 1), (0, 5), (0, 9)]):
            add(si, kt, (lambda jj: lambda: proj_qk(wk_sb, kT_sb, 0,
                                                    (jj + 1) * 512))(j))
        for j, (si, kt) in enumerate([(0, 11), (0, 14), (1, 2), (1, 5)]):
            add(si, kt, (lambda jj: lambda: proj_qk(wk_sb, kT_sb, 1,
                                                    jj * 512))(j))
        # v projections: 2 upfront, then one per kt in sweep 0
        for t in range(2, TT):
            add(0, t - 2, (lambda tt: lambda: proj_v(tt))(t))
        # q projections: unit for sweep si+2 emitted at (si, kt=6)
        for si in range(2, len(sweeps)):
            tq, tp = sweeps[si]
            add(si - 2, 6, (lambda a, b: lambda: proj_qk(
                wq_sb, qT_sb, b, a * QB))(tq, tp))
        # sweep 1 needs q(m1, qb0) early: emitted in sweep 0
        add(0, 10, lambda: proj_qk(wq_sb, qT_sb, 1, 0))

        # upfront minimal projections
        proj_qk(wk_sb, kT_sb, 0, 0)
        proj_qk(wq_sb, qT_sb, 0, 0)
        proj_v(0)
        proj_v(1)

        fifo = []
        due = []   # ring-consuming side work, emitted only when pend is
                   # empty so exp runs keep contiguous ring slots

        def drain_due():
            if not pend:
                while due:
                    due.pop(0)()

        for si, (qb, pair) in enumerate(sweeps):
            for kt in range(TT):
                due.extend(prework[(si, kt)])
                drain_due()
                ppA, ppB = scores_pair(pair, qb, kt)
                drain_due()
                if len(fifo) >= LAG:
                    attnv(*fifo.pop(0))
                fifo.append((pair, kt, ppA, ppB))
                # sweep-end bookkeeping: emitted right after the previous
                # sweep's final attnv pop (kt==LAG-1) and before the pop at
                # kt==LAG opens the next accumulation group in those banks
                if kt == LAG - 1 and si > 0:
                    pqb, ppair = sweeps[si - 1]
                    normalize(pqb, ppair)
                    if ppair == 1:
                        for i in range(4):
                            due.append((lambda a: lambda: emerge_t(a))(
                                4 * pqb + i))
        flush_exp()
        while fifo:
            attnv(*fifo.pop(0))
        normalize(*sweeps[-1])
        for i in range(4):
            emerge_t(TT - 4 + i)

    nc.compile()
    return nc


def prep_inputs(x, Wq, Wk, Wv, We, beta, input_valve, output_valve, chi,
                cfg=CFG):
    """Host-side prep: fold chamber into We, scale weights for fp8, shard."""
    x = np.asarray(x, np.float32)
    Wq = np.asarray(Wq, np.float32)
    Wk = np.asarray(Wk, np.float32)
    Wv = np.asarray(Wv, np.float32)
    We = np.asarray(We, np.float32)

    def sig(v):
        return 1.0 / (1.0 + np.exp(-np.asarray(v, np.float64)))

    b = sig(beta)
    iv = sig(input_valve)
    ov = sig(output_valve)
    g = np.tanh(np.asarray(chi, np.float64))
    ang = math.pi * b
    ca, sa = np.cos(ang), np.sin(ang)
    half = DH // 2

    We64 = We.astype(np.float64)
    WeP = np.empty((D, D), np.float64)
    for h in range(H):
        L = np.zeros((DH, DH))
        idx = np.arange(half)
        L[idx, idx] = ca[h]
        L[idx, half + idx] = -sa[h]
        L[half + idx, idx] = sa[h]
        L[half + idx, half + idx] = ca[h]
        L *= ov[h] * g[h] * iv[h]
        WeP[:, h * DH:(h + 1) * DH] = We64[:, h * DH:(h + 1) * DH] @ L

    bf = _np_dt("bfloat16")
    WqT = np.ascontiguousarray(Wq.T).astype(bf)
    WkT = np.ascontiguousarray(Wk.T).astype(bf)
    WvT = np.ascontiguousarray(Wv.T).astype(bf)
    WeT = np.ascontiguousarray(WeP.T).astype(bf)   # [c, dout]

    in_maps = []
    for core in range(NCORES):
        bidx, grp = divmod(core, H // HC)
        cols = slice(grp * C, (grp + 1) * C)
        in_maps.append({
            "xt": np.ascontiguousarray(x[bidx].T).astype(bf),
            "wq": np.ascontiguousarray(WqT[:, cols]),
            "wk": np.ascontiguousarray(WkT[:, cols]),
            "wv": np.ascontiguousarray(WvT[:, cols]),
            "we": np.ascontiguousarray(WeT[cols, :]),
        })
    return in_maps


def kernel(**inputs):
    global LAST_EXEC_NS
    import os
    if "nc" not in _CACHE:
        _CACHE["nc"] = build_nc()
    nc = _CACHE["nc"]
    in_maps = prep_inputs(**inputs)

    from concourse.bass_utils import run_bass_kernel_spmd
    trace = bool(os.environ.get("CIRC_TRACE"))
    res = run_bass_kernel_spmd(nc, in_maps, list(range(NCORES)), trace=trace)
    LAST_EXEC_NS = res.exec_time_ns
    _CACHE["last_results"] = res

    B = 2
    outp = np.zeros((B, T, D), np.float32)
    per_batch = NCORES // B
    for core in range(NCORES):
        outp[core // per_batch] += res.results[core]["out"]
    return outp
